# revision 33
# baseline (speedup 1.0000x reference)
"""CheckersGPT dense transformer forward pass on 8 Trainium2 NeuronCores.

Strategy: pure data-parallel over the batch dim (16 batches -> 2 per core),
plus host-side weight folding that removes ~40% of the matmul flops:

  M_h   = wq_h @ wk_h.T        energy = (x M_h) x^T  -- K projection gone
  wvo_h = wv_h @ wo[hE:(h+1)E]  attn  = sum_h att_h (x wvo_h) -- wo stage gone

Per layer / head: GT = M_h^T-projected x^T (serves as the energy lhsT; x^T
itself serves as the K^T operand), ZN = x @ wvo_h. att^T @ ZN accumulates
over all 8 heads directly in PSUM, so there is no per-head output-projection
or SBUF accumulation. The last layer only needs the final token of each
batch: energy = (x_last M) x^T and out = sum_h (att_h x) wvo_h -- all tiny.

Numerics: matmul operands are bf16 (weights pre-folded + converted on host;
activations rounded to bf16 on PSUM->SBUF evacuation), accumulation is fp32
in PSUM, softmax / layernorm / residual math is fp32. Softmax skips the
max-subtraction: energies are bounded (|e| < ~50 for this model) and exp is
computed in fp32 where overflow needs e > 88.

Layout per core (P=128 partitions):
  xT   [128, 4, 512]  : x^T; chunk c holds embed dims [128c,128c+128),
                        free dim = 512 tokens (2 batches x 256).
  xN   [128, 4, 512]  : x natural fp32; chunk c holds tokens [128c,..+128),
                        free dim = 512 embed. Residual / LN stream.
  xNb  [128, 4, 512]  : bf16 copy of xN (transpose source, last-layer rhs).
All matmuls are out = lhsT.T @ rhs contracting over the partition dim.
"""

import os
import numpy as np
from contextlib import ExitStack

import ml_dtypes
import concourse.bass as bass
import concourse.tile as tile
from concourse import bacc, mybir
from concourse.bass_utils import run_bass_kernel_spmd

F32 = mybir.dt.float32
BF16 = mybir.dt.bfloat16
I32 = mybir.dt.int32
AX = mybir.AxisListType
ALU = mybir.AluOpType
ACTF = mybir.ActivationFunctionType

V, E, L, H, B, T = 512, 512, 6, 8, 16, 256
NCORES = 8
BPC = B // NCORES          # batches per core
TOK = BPC * T              # tokens per core
P = 128
EC = E // P                # embed chunks of 128
TC = TOK // P              # token chunks of 128
NEG = -1e9
EPS = 1e-5

MODE = os.environ.get("CKGPT_MM_DT", "bf16")   # bf16 | f32r | f32
MM_DT = {"bf16": BF16, "f32r": F32, "f32": F32}[MODE]
MM_CAST = mybir.dt.float32r if MODE == "f32r" else None
NP_WDT = ml_dtypes.bfloat16 if MODE == "bf16" else np.float32

_CACHE = {}


def _c(ap):
    """Cast an AP for matmul input (f32r mode only)."""
    return ap.bitcast(MM_CAST) if MM_CAST is not None else ap


def _mm(nc, out, lhsT, rhs, start, stop):
    nc.tensor.matmul(out, _c(lhsT), _c(rhs), start=start, stop=stop)


def _build(nlayers=L, reps=1, last_opt=True, plain=True):
    """plain=True compiles for the model as generated by setup_inputs():
    all biases zero and layernorm weights/biases identity, so those ops are
    skipped. kernel() checks the actual inputs and falls back to the general
    variant if they are not."""
    nc = bacc.Bacc("TRN2", target_bir_lowering=False, debug=False, num_devices=NCORES)

    def din(name, shape, dtype=F32):
        return nc.dram_tensor(name, list(shape), dtype, kind="ExternalInput").ap()

    tok = din("tok", [P, TC], I32)            # token ids, p-major within chunks
    emb = din("emb", [V, E])
    pe2 = din("pe2", [TOK, E])                # positional encoding tiled over BPC
    mqk = din("mqk", [L, H, E, E], MM_DT)     # wq @ wk.T
    wvo = din("wvo", [L, H, E, E], MM_DT)     # wv @ wo_h
    bo = din("bo", [L, E])
    ln1w = din("ln1w", [L, E])
    ln1b = din("ln1b", [L, E])
    ln2w = din("ln2w", [L, E])
    ln2b = din("ln2b", [L, E])
    ff1w = din("ff1w", [L, E, E], MM_DT)
    ff1b = din("ff1b", [L, E])
    ff2w = din("ff2w", [L, E, E], MM_DT)
    ff2b = din("ff2b", [L, E])
    wout = din("wout", [E, V], MM_DT)
    bout = din("bout", [V])
    masks = din("masks", [P, 3 * P])          # packed causal mask [i0|j0, i1|j0, i1|j1]
    ident = din("ident", [P, P])
    probs = nc.dram_tensor("probs", [BPC, V], F32, kind="ExternalOutput").ap()
    aps = (emb, pe2, mqk, wvo, bo, ln1w, ln1b, ln2w, ln2b,
           ff1w, ff1b, ff2w, ff2b, wout, bout, masks, ident, probs, tok)

    with tile.TileContext(nc) as tc, ExitStack() as ctx:
        if reps > 1:
            with tc.For_i(0, reps, 1):
                _emit(nc, tc, ctx, aps, nlayers, last_opt, plain)
        else:
            _emit(nc, tc, ctx, aps, nlayers, last_opt, plain)

    nc.compile()
    return nc


def _emit(nc, tc, ctx, aps, nlayers, last_opt, plain):
    (emb, pe2, mqk, wvo, bo, ln1w, ln1b, ln2w, ln2b,
     ff1w, ff1b, ff2w, ff2b, wout, bout, masks, ident, probs, tok) = aps
    ep = ctx.enter_context

    const = ep(tc.tile_pool(name="const", bufs=1))
    w_p = ep(tc.tile_pool(name="wp", bufs=4))
    wff_p = ep(tc.tile_pool(name="wff", bufs=1))
    bias_p = ep(tc.tile_pool(name="bias", bufs=1))
    act_p = ep(tc.tile_pool(name="act", bufs=1))
    gz_p = ep(tc.tile_pool(name="gz", bufs=1))
    att_p = ep(tc.tile_pool(name="attp", bufs=1))
    ff_p = ep(tc.tile_pool(name="ffact", bufs=1))
    tmp_p = ep(tc.tile_pool(name="tmp", bufs=2))
    esb_p = ep(tc.tile_pool(name="esb", bufs=3))
    st_p = ep(tc.tile_pool(name="stats", bufs=4))
    out_p = ep(tc.tile_pool(name="outp", bufs=1))

    ppb = ep(tc.tile_pool(name="ppb", bufs=3, space="PSUM"))
    ppa = ep(tc.tile_pool(name="ppa", bufs=3, space="PSUM"))
    ppt = ep(tc.tile_pool(name="ppt", bufs=2, space="PSUM"))

    # ---- constants ----
    ident_t = const.tile([P, P], F32)
    nc.sync.dma_start(out=ident_t[:], in_=ident)
    ident_b = const.tile([P, P], BF16)
    nc.scalar.copy(ident_b[:], ident_t[:])
    mask_t = const.tile([P, 3 * P], F32)
    nc.sync.dma_start(out=mask_t[:], in_=masks)
    eps_t = const.tile([P, 1], F32)
    nc.vector.memset(eps_t[:], EPS)
    tok_t = const.tile([P, TC], I32)
    nc.sync.dma_start(out=tok_t[:], in_=tok)

    def wtile(pool, dram2d, tag):
        t = pool.tile([P, EC, E], MM_DT, tag=tag)
        nc.sync.dma_start(
            out=_c(t[:]),
            in_=_c(dram2d.rearrange("(c p) o -> p c o", p=P)),
        )
        return t

    def bbcast(vec_ap, tag="bias"):
        t = bias_p.tile([P, E], F32, tag=tag)
        nc.sync.dma_start(out=t[:], in_=vec_ap.partition_broadcast(P))
        return t

    def evac(dst, src, use_act):
        """PSUM -> SBUF copy (dtype conversion happens on write)."""
        if use_act:
            nc.scalar.copy(_c(dst), src)
        else:
            nc.vector.tensor_copy(_c(dst), src)

    def transpose_chunk_bf(dstT, srcN, a):
        # dstT[:, bb, a*P:(a+1)*P] = srcN[:, a, bb*P:(bb+1)*P].T  (bf16)
        for bb in range(EC):
            tp = ppt.tile([P, P], BF16, tag="tp")
            nc.tensor.transpose(
                tp[:], srcN[:, a, bb * P : (bb + 1) * P], ident_b[:]
            )
            evac(dstT[:, bb, a * P : (a + 1) * P], tp[:], (a + bb) % 2)

    def transpose_into_bf(dstT, srcN):
        for a in range(TC):
            transpose_chunk_bf(dstT, srcN, a)

    def layernorm(src, dst, w_b, b_b, tag, rows=P):
        # dst = (src - mean)/sqrt(var+eps) * w + b ; src [rows, E] fp32
        stt = st_p.tile([P, 6], F32, tag=tag + "s")
        nc.vector.bn_stats(out=stt[:rows], in_=src)
        mv = st_p.tile([P, 2], F32, tag=tag + "m")
        nc.vector.bn_aggr(out=mv[:rows], in_=stt[:rows])
        rs = st_p.tile([P, 1], F32, tag=tag + "r")
        nc.scalar.activation(
            out=rs[:rows], in_=mv[:rows, 1:2], func=ACTF.Abs_reciprocal_sqrt,
            bias=eps_t[:rows, 0:1],
        )
        if plain:
            nc.vector.tensor_scalar(
                out=dst, in0=src, scalar1=mv[:rows, 0:1], scalar2=rs[:rows],
                op0=ALU.subtract, op1=ALU.mult,
            )
        else:
            t = tmp_p.tile([P, E], F32, tag="lnt")
            nc.vector.tensor_scalar(
                out=t[:rows, :], in0=src, scalar1=mv[:rows, 0:1], scalar2=rs[:rows],
                op0=ALU.subtract, op1=ALU.mult,
            )
            nc.gpsimd.tensor_mul(out=t[:rows, :], in0=t[:rows, :], in1=w_b[:rows, :])
            nc.gpsimd.tensor_add(out=dst, in0=t[:rows, :], in1=b_b[:rows, :])

    # ---- embedding gather + positional encoding ----
    xN = act_p.tile([P, TC, E], F32, tag="xN")
    for c in range(TC):
        nc.gpsimd.indirect_dma_start(
            out=xN[:, c, :], out_offset=None, in_=emb,
            in_offset=bass.IndirectOffsetOnAxis(ap=tok_t[:, c : c + 1], axis=0),
        )
    pe_t = act_p.tile([P, TC, E], F32, tag="acc", bufs=2)
    nc.sync.dma_start(out=pe_t[:], in_=pe2.rearrange("(c p) o -> p c o", p=P))
    xNb = act_p.tile([P, TC, E], BF16, tag="xNb")
    xT = act_p.tile([P, EC, TOK], MM_DT, tag="xT")
    for c in range(TC):
        nc.vector.tensor_add(out=xN[:, c, :], in0=xN[:, c, :], in1=pe_t[:, c, :])
        if c % 2:
            nc.scalar.copy(xNb[:, c, :], xN[:, c, :])
        else:
            nc.gpsimd.tensor_copy(xNb[:, c, :], xN[:, c, :])
        transpose_chunk_bf(xT, xNb, c)

    for l in range(nlayers):
        last = last_opt and (l == L - 1) and (nlayers == L)
        if not plain:
            bo_b = bbcast(bo[l], "b_bo")
            ln1w_b = bbcast(ln1w[l], "b_l1w")
            ln1b_b = bbcast(ln1b[l], "b_l1b")
            ln2w_b = bbcast(ln2w[l], "b_l2w")
            ln2b_b = bbcast(ln2b[l], "b_l2b")
            ff2b_b = bbcast(ff2b[l], "b_f2")
            ff1b_t = bias_p.tile([P, EC], F32, tag="b_f1")
            nc.sync.dma_start(
                out=ff1b_t[:], in_=ff1b[l].rearrange("(c p) -> p c", p=P)
            )
        else:
            bo_b = ln1w_b = ln1b_b = ln2w_b = ln2b_b = ff2b_b = ff1b_t = None

        if not last:
            # ---- phase A: projections for all heads, then all energies ----
            GT_all = gz_p.tile([P, H, EC, TOK], MM_DT, tag="GT")
            ZN_all = gz_p.tile([P, H, TC, E], MM_DT, tag="ZN")
            attbf = att_p.tile([P, H, BPC, 3 * P], MM_DT, tag="attbf")

            for h in range(H):
                m_t = wtile(w_p, mqk[l, h], "w")
                # GT[o, t] = sum_e M[e,o] xT[e,t]
                for oc in range(EC):
                    ps = ppb.tile([P, TOK], F32, tag="ppb")
                    for ec in range(EC):
                        _mm(nc, ps[:], m_t[:, ec, oc * P : (oc + 1) * P],
                            xT[:, ec, :], ec == 0, ec == EC - 1)
                    evac(GT_all[:, h, oc, :], ps[:], oc % 2)
            for h in range(H):
                wvo_t = wtile(w_p, wvo[l, h], "w")
                # ZN[t, o] = sum_e x[t,e] wvo[e,o]
                for tcc in range(TC):
                    ps = ppb.tile([P, E], F32, tag="ppb")
                    for ec in range(EC):
                        _mm(nc, ps[:], xT[:, ec, tcc * P : (tcc + 1) * P],
                            wvo_t[:, ec, :], ec == 0, ec == EC - 1)
                    evac(ZN_all[:, h, tcc, :], ps[:], tcc % 2)
            # energies + softmax: pse packs [i0|j0 , i1|j0 , i1|j1]
            for h in range(H):
                for b in range(BPC):
                    t0 = b * T
                    pse = ppa.tile([P, 3 * P], F32, tag="ppa")
                    for oc in range(EC):
                        _mm(nc, pse[:, 0:P],
                            GT_all[:, h, oc, t0 : t0 + P],
                            xT[:, oc, t0 : t0 + P], oc == 0, oc == EC - 1)
                    for oc in range(EC):
                        _mm(nc, pse[:, P : 3 * P],
                            GT_all[:, h, oc, t0 + P : t0 + T],
                            xT[:, oc, t0 : t0 + T], oc == 0, oc == EC - 1)
                    att = esb_p.tile([P, 3 * P], F32, tag="esb")
                    nc.vector.tensor_add(out=att[:], in0=pse[:], in1=mask_t[:])
                    den = st_p.tile([P, 2], F32, tag="den")
                    nc.scalar.activation(
                        out=att[:, 0:P], in_=att[:, 0:P], func=ACTF.Exp,
                        accum_out=den[:, 0:1],
                    )
                    nc.scalar.activation(
                        out=att[:, P : 3 * P], in_=att[:, P : 3 * P], func=ACTF.Exp,
                        accum_out=den[:, 1:2],
                    )
                    rec = st_p.tile([P, 2], F32, tag="rec")
                    nc.vector.reciprocal(out=rec[:], in_=den[:])
                    nc.vector.tensor_scalar_mul(
                        out=_c(attbf[:, h, b, 0:P]), in0=att[:, 0:P],
                        scalar1=rec[:, 0:1],
                    )
                    nc.gpsimd.tensor_scalar_mul(
                        out=_c(attbf[:, h, b, P : 3 * P]), in0=att[:, P : 3 * P],
                        scalar1=rec[:, 1:2],
                    )

            # ---- phases T+B fused per token-chunk: transpose the att blocks
            # this chunk needs, run its head-accumulated att^T @ ZN chain,
            # evacuate + LN1 it, then (one chunk behind) its h1T transposes,
            # so LN latency hides under the next chunk's matmul work.
            attT = att_p.tile([P, H, BPC, 3, P], MM_DT, tag="attT")
            attn_acc = act_p.tile([P, TC, E], F32, tag="acc", bufs=2)
            h1N = ff_p.tile([P, TC, E], MM_DT, tag="h1N")
            h1T = ff_p.tile([P, EC, TOK], MM_DT, tag="ffT1")
            k = 0
            for tcc in range(TC):
                b, loc = divmod(tcc, 2)
                for blk in ((0,) if loc == 0 else (1, 2)):
                    for h in range(H):
                        tp = ppt.tile([P, P], MM_DT, tag="tp")
                        nc.tensor.transpose(
                            tp[:], _c(attbf[:, h, b, blk * P : (blk + 1) * P]),
                            _c(ident_b[:]) if MM_CAST is None and MODE == "bf16"
                            else ident_t[:],
                        )
                        evac(_c(attT[:, h, b, blk, :]), tp[:], k % 2)
                        k += 1
                acc = ppb.tile([P, E], F32, tag="ppb")
                if loc == 0:
                    for h in range(H):
                        _mm(nc, acc[:], attT[:, h, b, 0, :],
                            ZN_all[:, h, 2 * b, :], h == 0, h == H - 1)
                else:
                    for h in range(H):
                        _mm(nc, acc[:], attT[:, h, b, 1, :],
                            ZN_all[:, h, 2 * b, :], h == 0, False)
                        _mm(nc, acc[:], attT[:, h, b, 2, :],
                            ZN_all[:, h, 2 * b + 1, :], False, h == H - 1)
                if plain:
                    nc.scalar.copy(attn_acc[:, tcc, :], acc[:])
                else:
                    nc.vector.tensor_add(
                        out=attn_acc[:, tcc, :], in0=acc[:], in1=bo_b[:]
                    )
                s1 = tmp_p.tile([P, E], F32, tag="s1")
                nc.vector.tensor_add(
                    out=s1[:], in0=attn_acc[:, tcc, :], in1=xN[:, tcc, :]
                )
                layernorm(s1[:], _c(h1N[:, tcc, :]), ln1w_b, ln1b_b, "ln1")
                if tcc > 0:
                    transpose_chunk_bf(h1T, h1N, tcc - 1)
            transpose_chunk_bf(h1T, h1N, TC - 1)
        else:
            # ---- last layer: only the final token of each batch ----
            # GTl[o, (h, b)] = sum_e M_h[e, o] x_last_b[e]
            GTl = att_p.tile([P, EC, H, BPC], MM_DT, tag="gtl")
            for h in range(H):
                m_t = wtile(w_p, mqk[l, h], "w")
                for oc in range(EC):
                    ps = ppt.tile([P, BPC], F32, tag="tp")
                    for ec in range(EC):
                        _mm(nc, ps[:], m_t[:, ec, oc * P : (oc + 1) * P],
                            xT[:, ec, T - 1 :: T], ec == 0, ec == EC - 1)
                    evac(GTl[:, oc, h, :], ps[:], (h + oc) % 2)
            # energy for all heads at once: [H, T] per batch
            WTt = att_p.tile([P, EC, H, BPC], MM_DT, tag="wtt")
            for b in range(BPC):
                t0 = b * T
                pse = ppa.tile([P, T], F32, tag="ppa")
                for oc in range(EC):
                    _mm(nc, pse[0:H, :], GTl[:, oc, :, b],
                        xT[:, oc, t0 : t0 + T], oc == 0, oc == EC - 1)
                attl = esb_p.tile([P, T], F32, tag="esb", bufs=3)
                denl = st_p.tile([P, 1], F32, tag="den0")
                nc.scalar.activation(
                    out=attl[0:H, :], in_=pse[0:H, :], func=ACTF.Exp,
                    accum_out=denl[0:H],
                )
                recl = st_p.tile([P, 1], F32, tag="rec0")
                nc.vector.reciprocal(out=recl[0:H], in_=denl[0:H])
                attlb = esb_p.tile([P, T], MM_DT, tag="esbl", bufs=1)
                nc.vector.tensor_scalar_mul(
                    out=_c(attlb[0:H, :]), in0=attl[0:H, :], scalar1=recl[0:H]
                )
                # attTl[j, h] per j-chunk
                attTl = esb_p.tile([P, 2, H], MM_DT, tag="attl", bufs=1)
                for jc in range(2):
                    tp = ppt.tile([P, H], MM_DT, tag="tp")
                    nc.tensor.transpose(
                        tp[:], _c(attlb[0:H, jc * P : (jc + 1) * P]),
                        _c(ident_b[0:H, 0:H]) if MODE == "bf16"
                        else ident_t[0:H, 0:H],
                    )
                    evac(attTl[:, jc, :], tp[:], jc % 2)
                # W[h, e] = sum_j att[h, j] x_b[j, e]
                wps = ppb.tile([P, E], F32, tag="ppb")
                _mm(nc, wps[0:H, :], attTl[:, 0, :], xNb[:, 2 * b, :], True, False)
                _mm(nc, wps[0:H, :], attTl[:, 1, :], xNb[:, 2 * b + 1, :], False, True)
                wbf = esb_p.tile([P, E], MM_DT, tag="wbf", bufs=1)
                nc.scalar.copy(_c(wbf[0:H, :]), wps[0:H, :])
                # WTt[e, h] per e-chunk (lhsT for the wvo projection)
                for ec in range(EC):
                    tp = ppt.tile([P, H], MM_DT, tag="tp")
                    nc.tensor.transpose(
                        tp[:], _c(wbf[0:H, ec * P : (ec + 1) * P]),
                        _c(ident_b[0:H, 0:H]) if MODE == "bf16"
                        else ident_t[0:H, 0:H],
                    )
                    evac(WTt[:, ec, :, b], tp[:], ec % 2)
            # out[b, o] = sum_{h,e} W[b,h,e] wvo_h[e,o], PSUM-accumulated
            pw_last = ppb.tile([BPC, E], F32, tag="ppb")
            for h in range(H):
                wvo_t = wtile(w_p, wvo[l, h], "w")
                for ec in range(EC):
                    _mm(nc, pw_last[:], WTt[:, ec, h, :], wvo_t[:, ec, :],
                        h == 0 and ec == 0, h == H - 1 and ec == EC - 1)

        # ---- FFN block ----
        ff1w_t = wtile(wff_p, ff1w[l], "wff1")
        ff2w_t = wtile(wff_p, ff2w[l], "wff2")

        if not last:
            r1T = ff_p.tile([P, EC, TOK], MM_DT, tag="ffT2")
            for fc in range(EC):
                ps = ppb.tile([P, TOK], F32, tag="ppb")
                for ec in range(EC):
                    _mm(nc, ps[:], ff1w_t[:, ec, fc * P : (fc + 1) * P],
                        h1T[:, ec, :], ec == 0, ec == EC - 1)
                if plain:
                    nc.scalar.activation(
                        out=_c(r1T[:, fc, :]), in_=ps[:], func=ACTF.Relu
                    )
                else:
                    nc.scalar.activation(
                        out=_c(r1T[:, fc, :]), in_=ps[:], func=ACTF.Relu,
                        bias=ff1b_t[:, fc : fc + 1],
                    )

            xN_new = act_p.tile([P, TC, E], F32, tag="xN")
            xNb_new = act_p.tile([P, TC, E], BF16, tag="xNb")
            xT_new = act_p.tile([P, EC, TOK], MM_DT, tag="xT")
            for tcc in range(TC):
                ps = ppb.tile([P, E], F32, tag="ppb")
                for fc in range(EC):
                    _mm(nc, ps[:], r1T[:, fc, tcc * P : (tcc + 1) * P],
                        ff2w_t[:, fc, :], fc == 0, fc == EC - 1)
                s2 = tmp_p.tile([P, E], F32, tag="s1")
                if plain:
                    nc.vector.tensor_add(
                        out=s2[:], in0=ps[:], in1=attn_acc[:, tcc, :]
                    )
                else:
                    nc.vector.tensor_add(out=s2[:], in0=ps[:], in1=ff2b_b[:])
                    nc.gpsimd.tensor_add(
                        out=s2[:], in0=s2[:], in1=attn_acc[:, tcc, :]
                    )
                layernorm(s2[:], xN_new[:, tcc, :], ln2w_b, ln2b_b, "ln2")
                if tcc % 2:
                    nc.scalar.copy(xNb_new[:, tcc, :], xN_new[:, tcc, :])
                else:
                    nc.vector.tensor_copy(xNb_new[:, tcc, :], xN_new[:, tcc, :])
                transpose_chunk_bf(xT_new, xNb_new, tcc)
            xN = xN_new
            xNb = xNb_new
            xT = xT_new
        else:
            # ---- last layer FFN on 2 tokens only ----
            x_l = out_p.tile([BPC, E], F32)
            for b in range(BPC):
                nc.sync.dma_start(
                    out=x_l[b : b + 1, :], in_=xN[P - 1 : P, 2 * b + 1, :]
                )
            if plain:
                ao_l = out_p.tile([BPC, E], F32)
                nc.vector.tensor_copy(ao_l[:], pw_last[:])
            else:
                ao_l = out_p.tile([BPC, E], F32)
                nc.vector.tensor_add(
                    out=ao_l[:], in0=pw_last[:], in1=bo_b[:BPC, :]
                )
            s1 = out_p.tile([BPC, E], F32)
            nc.vector.tensor_add(out=s1[:], in0=ao_l[:], in1=x_l[:])
            h1_l = out_p.tile([BPC, E], F32)
            layernorm(s1[:], h1_l[:], ln1w_b, ln1b_b, "lnL1", rows=BPC)
            h1T_l = ff_p.tile([P, EC, BPC], MM_DT, tag="h1Tl")
            for bb in range(EC):
                tp = ppt.tile([P, BPC], F32, tag="tp")
                nc.tensor.transpose(
                    tp[:], h1_l[:, bb * P : (bb + 1) * P],
                    ident_t[0:BPC, 0:BPC],
                )
                evac(h1T_l[:, bb, :], tp[:], bb % 2)
            r1T_l = ff_p.tile([P, EC, BPC], MM_DT, tag="r1Tl")
            for fc in range(EC):
                ps = ppt.tile([P, BPC], F32, tag="tp")
                for ec in range(EC):
                    _mm(nc, ps[:], ff1w_t[:, ec, fc * P : (fc + 1) * P],
                        h1T_l[:, ec, :], ec == 0, ec == EC - 1)
                if plain:
                    nc.scalar.activation(
                        out=_c(r1T_l[:, fc, :]), in_=ps[:], func=ACTF.Relu
                    )
                else:
                    nc.scalar.activation(
                        out=_c(r1T_l[:, fc, :]), in_=ps[:], func=ACTF.Relu,
                        bias=ff1b_t[:, fc : fc + 1],
                    )
            ps2 = ppb.tile([BPC, E], F32, tag="ppb")
            for fc in range(EC):
                _mm(nc, ps2[:], r1T_l[:, fc, :], ff2w_t[:, fc, :],
                    fc == 0, fc == EC - 1)
            s2 = out_p.tile([BPC, E], F32)
            if not plain:
                nc.vector.tensor_add(out=s2[:], in0=ps2[:], in1=ff2b_b[:BPC, :])
                nc.vector.tensor_add(out=s2[:], in0=s2[:], in1=ao_l[:])
            else:
                nc.vector.tensor_add(out=s2[:], in0=ps2[:], in1=ao_l[:])
            xl = out_p.tile([BPC, E], F32)
            layernorm(s2[:], xl[:], ln2w_b, ln2b_b, "lnL2", rows=BPC)
            xlT = ff_p.tile([P, EC, BPC], MM_DT, tag="xlT")
            for bb in range(EC):
                tp = ppt.tile([P, BPC], F32, tag="tp")
                nc.tensor.transpose(
                    tp[:], xl[:, bb * P : (bb + 1) * P],
                    ident_t[0:BPC, 0:BPC],
                )
                evac(xlT[:, bb, :], tp[:], bb % 2)

    # ---- output head: last token of each batch ----
    wout_t = wtile(wff_p, wout, "wff1")
    pl = ppb.tile([BPC, V], F32, tag="ppb")
    if nlayers == L and last_opt:
        xl_lhs = xlT
        cols = slice(0, BPC)
    else:
        xl_lhs = xT
        cols = slice(T - 1, TOK, T)
    for ec in range(EC):
        _mm(nc, pl[:], xl_lhs[:, ec, cols], wout_t[:, ec, :], ec == 0, ec == EC - 1)
    logits = out_p.tile([BPC, V], F32)
    if plain:
        nc.vector.tensor_copy(logits[:], pl[:])
    else:
        bout_t = out_p.tile([BPC, V], F32)
        nc.sync.dma_start(out=bout_t[:], in_=bout.partition_broadcast(BPC))
        nc.vector.tensor_add(out=logits[:], in0=pl[:], in1=bout_t[:])
    nmax = out_p.tile([BPC, 1], F32)
    nc.vector.reduce_max(out=nmax[:], in_=logits[:], axis=AX.X, negate=True)
    den = out_p.tile([BPC, 1], F32)
    nc.scalar.activation(
        out=logits[:], in_=logits[:], func=ACTF.Exp,
        bias=nmax[:, 0:1], accum_out=den[:],
    )
    rec = out_p.tile([BPC, 1], F32)
    nc.vector.reciprocal(out=rec[:], in_=den[:])
    nc.vector.tensor_scalar_mul(out=logits[:], in0=logits[:], scalar1=rec[:])
    nc.sync.dma_start(out=probs, in_=logits[:])


def _pe_table():
    i = np.arange(E, dtype=np.float32)
    rates = (1.0 / np.power(np.float32(10000.0), 2.0 * np.floor(i / 2.0) / E)).astype(
        np.float32
    )
    ang = np.arange(T, dtype=np.float32)[:, None] * rates[None, :]
    pe = np.concatenate([np.sin(ang[:, 0::2]), np.cos(ang[:, 1::2])], axis=-1)
    return np.tile(pe.astype(np.float32), (BPC, 1))  # [TOK, E]


def _masks():
    # packed additive mask [P, 3*P]: block0 = (i0, j0), block1 = (i1, j0),
    # block2 = (i1, j1);  mask[i, j] = NEG where j > i
    m = np.zeros((P, 3 * P), dtype=np.float32)
    p = np.arange(P)
    m[:, 0:P] = np.where(p[None, :] > p[:, None], np.float32(NEG), 0.0)       # i0,j0
    # block1: i in [128,256), j in [0,128): j <= 127 < 128 <= i, never masked
    m[:, 2 * P : 3 * P] = np.where(p[None, :] > p[:, None], np.float32(NEG), 0.0)  # i1,j1
    return m


def _prep_in_maps(
    input_tokens, emb, wq, wk, wv, wo, bo, ln1_w, ln1_b, ln2_w, ln2_b,
    ff1_w, ff1_b, ff2_w, ff2_b, wout, bout,
):
    f = lambda x: np.ascontiguousarray(np.asarray(x, dtype=np.float32))
    w = lambda x: np.ascontiguousarray(np.asarray(x, dtype=np.float32).astype(NP_WDT))
    toks = np.asarray(input_tokens).astype(np.int64)
    wq32 = np.asarray(wq, dtype=np.float32)
    wk32 = np.asarray(wk, dtype=np.float32)
    wv32 = np.asarray(wv, dtype=np.float32)
    wo32 = np.asarray(wo, dtype=np.float32).reshape(L, H, E, E)
    mqk = np.matmul(wq32, wk32.transpose(0, 1, 3, 2))   # [L,H,E,E]
    wvo = np.matmul(wv32, wo32)                         # [L,H,E,E]
    shared = {
        "emb": f(emb), "mqk": w(mqk), "wvo": w(wvo),
        "bo": f(bo), "ln1w": f(ln1_w), "ln1b": f(ln1_b), "ln2w": f(ln2_w),
        "ln2b": f(ln2_b), "ff1w": w(ff1_w), "ff1b": f(ff1_b), "ff2w": w(ff2_w),
        "ff2b": f(ff2_b), "wout": w(wout), "bout": f(bout),
        "pe2": _pe_table(), "masks": _masks(),
        "ident": np.eye(P, dtype=np.float32),
    }
    in_maps = []
    for c in range(NCORES):
        t = toks[c * BPC : (c + 1) * BPC].reshape(TOK)  # [512] flat tokens
        tokarr = np.ascontiguousarray(t.reshape(TC, P).T.astype(np.int32))
        in_maps.append({**shared, "tok": tokarr})
    return in_maps


def _inputs_are_plain(inputs):
    z = lambda k: not np.any(np.asarray(inputs[k], dtype=np.float32))
    o = lambda k: np.all(np.asarray(inputs[k], dtype=np.float32) == 1.0)
    return (z("bo") and z("ln1_b") and z("ln2_b") and z("ff1_b") and z("ff2_b")
            and z("bout") and o("ln1_w") and o("ln2_w"))


def kernel(**inputs):
    plain = _inputs_are_plain(inputs)
    key = f"nc_{plain}"
    if key not in _CACHE:
        _CACHE[key] = _build(plain=plain)
    nc = _CACHE[key]
    in_maps = _prep_in_maps(**inputs)
    res = run_bass_kernel_spmd(nc, in_maps, core_ids=list(range(NCORES)))
    _CACHE["last_results"] = res
    out = np.concatenate([res.results[c]["probs"] for c in range(NCORES)], axis=0)
    return out.astype(np.float32)


# revision 36
# speedup vs baseline: 5.1069x; 5.1069x over previous
"""CheckersGPT dense transformer forward pass on 8 Trainium2 NeuronCores.

Strategy: pure data-parallel over the batch dim (16 batches -> 2 per core),
plus host-side weight folding that removes ~40% of the matmul flops:

  M_h   = wq_h @ wk_h.T        energy = (x M_h) x^T  -- K projection gone
  wvo_h = wv_h @ wo[hE:(h+1)E]  attn  = sum_h att_h (x wvo_h) -- wo stage gone

Per layer / head: GT = M_h^T-projected x^T (serves as the energy lhsT; x^T
itself serves as the K^T operand), ZN = x @ wvo_h. att^T @ ZN accumulates
over all 8 heads directly in PSUM, so there is no per-head output-projection
or SBUF accumulation. The last layer only needs the final token of each
batch: energy = (x_last M) x^T and out = sum_h (att_h x) wvo_h -- all tiny.

Numerics: matmul operands are bf16 (weights pre-folded + converted on host;
activations rounded to bf16 on PSUM->SBUF evacuation), accumulation is fp32
in PSUM, softmax / layernorm / residual math is fp32. Softmax skips the
max-subtraction: energies are bounded (|e| < ~50 for this model) and exp is
computed in fp32 where overflow needs e > 88.

Layout per core (P=128 partitions):
  xT   [128, 4, 512]  : x^T; chunk c holds embed dims [128c,128c+128),
                        free dim = 512 tokens (2 batches x 256).
  xN   [128, 4, 512]  : x natural fp32; chunk c holds tokens [128c,..+128),
                        free dim = 512 embed. Residual / LN stream.
  xNb  [128, 4, 512]  : bf16 copy of xN (transpose source, last-layer rhs).
All matmuls are out = lhsT.T @ rhs contracting over the partition dim.
"""

import os
import numpy as np
from contextlib import ExitStack

import ml_dtypes
import concourse.bass as bass
import concourse.tile as tile
from concourse import bacc, mybir
from concourse.bass_utils import run_bass_kernel_spmd

F32 = mybir.dt.float32
BF16 = mybir.dt.bfloat16
I32 = mybir.dt.int32
AX = mybir.AxisListType
ALU = mybir.AluOpType
ACTF = mybir.ActivationFunctionType

V, E, L, H, B, T = 512, 512, 6, 8, 16, 256
NCORES = 8
BPC = B // NCORES          # batches per core
TOK = BPC * T              # tokens per core
P = 128
EC = E // P                # embed chunks of 128
TC = TOK // P              # token chunks of 128
NEG = -1e9
EPS = 1e-5

MODE = os.environ.get("CKGPT_MM_DT", "bf16")   # bf16 | f32r | f32
MM_DT = {"bf16": BF16, "f32r": F32, "f32": F32}[MODE]
MM_CAST = mybir.dt.float32r if MODE == "f32r" else None
NP_WDT = ml_dtypes.bfloat16 if MODE == "bf16" else np.float32

_CACHE = {}


def _c(ap):
    """Cast an AP for matmul input (f32r mode only)."""
    return ap.bitcast(MM_CAST) if MM_CAST is not None else ap


def _mm(nc, out, lhsT, rhs, start, stop):
    nc.tensor.matmul(out, _c(lhsT), _c(rhs), start=start, stop=stop)


def _build(nlayers=L, reps=1, last_opt=True, plain=True):
    """plain=True compiles for the model as generated by setup_inputs():
    all biases zero and layernorm weights/biases identity, so those ops are
    skipped. kernel() checks the actual inputs and falls back to the general
    variant if they are not."""
    nc = bacc.Bacc("TRN2", target_bir_lowering=False, debug=False, num_devices=NCORES)

    def din(name, shape, dtype=F32):
        return nc.dram_tensor(name, list(shape), dtype, kind="ExternalInput").ap()

    tok = din("tok", [P, TC], I32)            # token ids, p-major within chunks
    emb = din("emb", [V, E])
    pe2 = din("pe2", [TOK, E])                # positional encoding tiled over BPC
    mqk = din("mqk", [L, H, E, E], MM_DT)     # wq @ wk.T
    wvo = din("wvo", [L, H, E, E], MM_DT)     # wv @ wo_h
    bo = din("bo", [L, E])
    ln1w = din("ln1w", [L, E])
    ln1b = din("ln1b", [L, E])
    ln2w = din("ln2w", [L, E])
    ln2b = din("ln2b", [L, E])
    ff1w = din("ff1w", [L, E, E], MM_DT)
    ff1b = din("ff1b", [L, E])
    ff2w = din("ff2w", [L, E, E], MM_DT)
    ff2b = din("ff2b", [L, E])
    wout = din("wout", [E, V], MM_DT)
    bout = din("bout", [V])
    masks = din("masks", [P, 3 * P])          # packed causal mask [i0|j0, i1|j0, i1|j1]
    ident = din("ident", [P, P])
    probs = nc.dram_tensor("probs", [BPC, V], F32, kind="ExternalOutput").ap()
    aps = (emb, pe2, mqk, wvo, bo, ln1w, ln1b, ln2w, ln2b,
           ff1w, ff1b, ff2w, ff2b, wout, bout, masks, ident, probs, tok)

    with tile.TileContext(nc) as tc, ExitStack() as ctx:
        if reps > 1:
            with tc.For_i(0, reps, 1):
                _emit(nc, tc, ctx, aps, nlayers, last_opt, plain)
        else:
            _emit(nc, tc, ctx, aps, nlayers, last_opt, plain)

    nc.compile()
    return nc


def _emit(nc, tc, ctx, aps, nlayers, last_opt, plain):
    (emb, pe2, mqk, wvo, bo, ln1w, ln1b, ln2w, ln2b,
     ff1w, ff1b, ff2w, ff2b, wout, bout, masks, ident, probs, tok) = aps
    ep = ctx.enter_context

    const = ep(tc.tile_pool(name="const", bufs=1))
    w_p = ep(tc.tile_pool(name="wp", bufs=4))
    wff_p = ep(tc.tile_pool(name="wff", bufs=1))
    bias_p = ep(tc.tile_pool(name="bias", bufs=1))
    act_p = ep(tc.tile_pool(name="act", bufs=1))
    gz_p = ep(tc.tile_pool(name="gz", bufs=1))
    att_p = ep(tc.tile_pool(name="attp", bufs=1))
    ff_p = ep(tc.tile_pool(name="ffact", bufs=1))
    tmp_p = ep(tc.tile_pool(name="tmp", bufs=2))
    esb_p = ep(tc.tile_pool(name="esb", bufs=3))
    st_p = ep(tc.tile_pool(name="stats", bufs=4))
    out_p = ep(tc.tile_pool(name="outp", bufs=1))

    ppb = ep(tc.tile_pool(name="ppb", bufs=3, space="PSUM"))
    ppa = ep(tc.tile_pool(name="ppa", bufs=3, space="PSUM"))
    ppt = ep(tc.tile_pool(name="ppt", bufs=2, space="PSUM"))

    # ---- constants ----
    ident_t = const.tile([P, P], F32)
    nc.sync.dma_start(out=ident_t[:], in_=ident)
    ident_b = const.tile([P, P], BF16)
    nc.scalar.copy(ident_b[:], ident_t[:])
    mask_t = const.tile([P, 3 * P], F32)
    nc.sync.dma_start(out=mask_t[:], in_=masks)
    eps_t = const.tile([P, 1], F32)
    nc.vector.memset(eps_t[:], EPS)
    tok_t = const.tile([P, TC], I32)
    nc.sync.dma_start(out=tok_t[:], in_=tok)

    def wtile(pool, dram2d, tag):
        t = pool.tile([P, EC, E], MM_DT, tag=tag)
        nc.sync.dma_start(
            out=_c(t[:]),
            in_=_c(dram2d.rearrange("(c p) o -> p c o", p=P)),
        )
        return t

    def bbcast(vec_ap, tag="bias"):
        t = bias_p.tile([P, E], F32, tag=tag)
        nc.sync.dma_start(out=t[:], in_=vec_ap.partition_broadcast(P))
        return t

    def evac(dst, src, use_act):
        """PSUM -> SBUF copy (dtype conversion happens on write)."""
        if use_act:
            nc.scalar.copy(_c(dst), src)
        else:
            nc.vector.tensor_copy(_c(dst), src)

    def transpose_chunk_bf(dstT, srcN, a):
        # dstT[:, bb, a*P:(a+1)*P] = srcN[:, a, bb*P:(bb+1)*P].T  (bf16)
        for bb in range(EC):
            tp = ppt.tile([P, P], BF16, tag="tp")
            nc.tensor.transpose(
                tp[:], srcN[:, a, bb * P : (bb + 1) * P], ident_b[:]
            )
            evac(dstT[:, bb, a * P : (a + 1) * P], tp[:], (a + bb) % 2)

    def transpose_into_bf(dstT, srcN):
        for a in range(TC):
            transpose_chunk_bf(dstT, srcN, a)

    def layernorm(src, dst, w_b, b_b, tag, rows=P):
        # dst = (src - mean)/sqrt(var+eps) * w + b ; src [rows, E] fp32
        stt = st_p.tile([P, 6], F32, tag=tag + "s")
        nc.vector.bn_stats(out=stt[:rows], in_=src)
        mv = st_p.tile([P, 2], F32, tag=tag + "m")
        nc.vector.bn_aggr(out=mv[:rows], in_=stt[:rows])
        sd = st_p.tile([P, 1], F32, tag=tag + "d")
        nc.scalar.activation(
            out=sd[:rows], in_=mv[:rows, 1:2], func=ACTF.Sqrt,
            bias=eps_t[:rows, 0:1],
        )
        rs = st_p.tile([P, 1], F32, tag=tag + "r")
        nc.vector.reciprocal(out=rs[:rows], in_=sd[:rows])
        if plain:
            nc.vector.tensor_scalar(
                out=dst, in0=src, scalar1=mv[:rows, 0:1], scalar2=rs[:rows],
                op0=ALU.subtract, op1=ALU.mult,
            )
        else:
            t = tmp_p.tile([P, E], F32, tag="lnt")
            nc.vector.tensor_scalar(
                out=t[:rows, :], in0=src, scalar1=mv[:rows, 0:1], scalar2=rs[:rows],
                op0=ALU.subtract, op1=ALU.mult,
            )
            nc.gpsimd.tensor_mul(out=t[:rows, :], in0=t[:rows, :], in1=w_b[:rows, :])
            nc.gpsimd.tensor_add(out=dst, in0=t[:rows, :], in1=b_b[:rows, :])

    # ---- embedding gather + positional encoding ----
    xN = act_p.tile([P, TC, E], F32, tag="xN")
    for c in range(TC):
        nc.gpsimd.indirect_dma_start(
            out=xN[:, c, :], out_offset=None, in_=emb,
            in_offset=bass.IndirectOffsetOnAxis(ap=tok_t[:, c : c + 1], axis=0),
        )
    pe_t = act_p.tile([P, TC, E], F32, tag="acc", bufs=2)
    nc.sync.dma_start(out=pe_t[:], in_=pe2.rearrange("(c p) o -> p c o", p=P))
    xNb = act_p.tile([P, TC, E], BF16, tag="xNb")
    xT = act_p.tile([P, EC, TOK], MM_DT, tag="xT")
    for c in range(TC):
        nc.vector.tensor_add(out=xN[:, c, :], in0=xN[:, c, :], in1=pe_t[:, c, :])
        if c % 2:
            nc.scalar.copy(xNb[:, c, :], xN[:, c, :])
        else:
            nc.gpsimd.tensor_copy(xNb[:, c, :], xN[:, c, :])
        transpose_chunk_bf(xT, xNb, c)

    for l in range(nlayers):
        last = last_opt and (l == L - 1) and (nlayers == L)
        if not plain:
            bo_b = bbcast(bo[l], "b_bo")
            ln1w_b = bbcast(ln1w[l], "b_l1w")
            ln1b_b = bbcast(ln1b[l], "b_l1b")
            ln2w_b = bbcast(ln2w[l], "b_l2w")
            ln2b_b = bbcast(ln2b[l], "b_l2b")
            ff2b_b = bbcast(ff2b[l], "b_f2")
            ff1b_t = bias_p.tile([P, EC], F32, tag="b_f1")
            nc.sync.dma_start(
                out=ff1b_t[:], in_=ff1b[l].rearrange("(c p) -> p c", p=P)
            )
        else:
            bo_b = ln1w_b = ln1b_b = ln2w_b = ln2b_b = ff2b_b = ff1b_t = None

        if not last:
            # ---- phase A: projections for all heads, then all energies ----
            GT_all = gz_p.tile([P, H, EC, TOK], MM_DT, tag="GT")
            ZN_all = gz_p.tile([P, H, TC, E], MM_DT, tag="ZN")
            attbf = att_p.tile([P, H, BPC, 3 * P], MM_DT, tag="attbf")

            for h in range(H):
                m_t = wtile(w_p, mqk[l, h], "w")
                wvo_t = wtile(w_p, wvo[l, h], "w")
                # GT[o, t] = sum_e M[e,o] xT[e,t]
                for oc in range(EC):
                    ps = ppb.tile([P, TOK], F32, tag="ppb")
                    for ec in range(EC):
                        _mm(nc, ps[:], m_t[:, ec, oc * P : (oc + 1) * P],
                            xT[:, ec, :], ec == 0, ec == EC - 1)
                    evac(GT_all[:, h, oc, :], ps[:], True)
                # ZN[t, o] = sum_e x[t,e] wvo[e,o]
                for tcc in range(TC):
                    ps = ppb.tile([P, E], F32, tag="ppb")
                    for ec in range(EC):
                        _mm(nc, ps[:], xT[:, ec, tcc * P : (tcc + 1) * P],
                            wvo_t[:, ec, :], ec == 0, ec == EC - 1)
                    evac(ZN_all[:, h, tcc, :], ps[:], False)
                # energies + softmax (i0 attends j0; i1 attends j0+j1)
                for b in range(BPC):
                    t0 = b * T
                    pse0 = ppa.tile([P, T], F32, tag="ppa")
                    for oc in range(EC):
                        _mm(nc, pse0[:, 0:P],
                            GT_all[:, h, oc, t0 : t0 + P],
                            xT[:, oc, t0 : t0 + P], oc == 0, oc == EC - 1)
                    pse1 = ppa.tile([P, T], F32, tag="ppa")
                    for oc in range(EC):
                        _mm(nc, pse1[:],
                            GT_all[:, h, oc, t0 + P : t0 + T],
                            xT[:, oc, t0 : t0 + T], oc == 0, oc == EC - 1)
                    att = esb_p.tile([P, 3 * P], F32, tag="esb")
                    nc.vector.tensor_add(
                        out=att[:, 0:P], in0=pse0[:, 0:P], in1=mask_t[:, 0:P]
                    )
                    nc.vector.tensor_add(
                        out=att[:, P : 3 * P], in0=pse1[:], in1=mask_t[:, P : 3 * P]
                    )
                    den = st_p.tile([P, 2], F32, tag="den")
                    nc.scalar.activation(
                        out=att[:, 0:P], in_=att[:, 0:P], func=ACTF.Exp,
                        accum_out=den[:, 0:1],
                    )
                    nc.scalar.activation(
                        out=att[:, P : 3 * P], in_=att[:, P : 3 * P], func=ACTF.Exp,
                        accum_out=den[:, 1:2],
                    )
                    rec = st_p.tile([P, 2], F32, tag="rec")
                    nc.vector.reciprocal(out=rec[:], in_=den[:])
                    nc.vector.tensor_scalar_mul(
                        out=_c(attbf[:, h, b, 0:P]), in0=att[:, 0:P],
                        scalar1=rec[:, 0:1],
                    )
                    nc.gpsimd.tensor_scalar_mul(
                        out=_c(attbf[:, h, b, P : 3 * P]), in0=att[:, P : 3 * P],
                        scalar1=rec[:, 1:2],
                    )

            # ---- phase T: transpose att blocks to [j, i] layout ----
            # blocks: 0 = (j0 -> i0), 1 = (j0 -> i1), 2 = (j1 -> i1)
            attT = att_p.tile([P, H, BPC, 3, P], MM_DT, tag="attT")
            k = 0
            for h in range(H):
                for b in range(BPC):
                    for blk in range(3):
                        tp = ppt.tile([P, P], MM_DT, tag="tp")
                        nc.tensor.transpose(
                            tp[:], _c(attbf[:, h, b, blk * P : (blk + 1) * P]),
                            _c(ident_b[:]) if MM_CAST is None and MODE == "bf16"
                            else ident_t[:],
                        )
                        evac(_c(attT[:, h, b, blk, :]), tp[:], k % 2)
                        k += 1

            # ---- phase B: att^T @ ZN accumulated over heads in PSUM ----
            attn_acc = act_p.tile([P, TC, E], F32, tag="acc", bufs=2)
            for tcc in range(TC):
                b, loc = divmod(tcc, 2)
                acc = ppb.tile([P, E], F32, tag="ppb")
                if loc == 0:
                    for h in range(H):
                        _mm(nc, acc[:], attT[:, h, b, 0, :],
                            ZN_all[:, h, 2 * b, :], h == 0, h == H - 1)
                else:
                    for h in range(H):
                        _mm(nc, acc[:], attT[:, h, b, 1, :],
                            ZN_all[:, h, 2 * b, :], h == 0, False)
                        _mm(nc, acc[:], attT[:, h, b, 2, :],
                            ZN_all[:, h, 2 * b + 1, :], False, h == H - 1)
                if plain:
                    nc.vector.tensor_copy(attn_acc[:, tcc, :], acc[:])
                else:
                    nc.vector.tensor_add(
                        out=attn_acc[:, tcc, :], in0=acc[:], in1=bo_b[:]
                    )

            h1N = ff_p.tile([P, TC, E], MM_DT, tag="h1N")
            h1T = ff_p.tile([P, EC, TOK], MM_DT, tag="ffT1")
            for tcc in range(TC):
                s1 = tmp_p.tile([P, E], F32, tag="s1")
                nc.gpsimd.tensor_add(
                    out=s1[:], in0=attn_acc[:, tcc, :], in1=xN[:, tcc, :]
                )
                layernorm(s1[:], _c(h1N[:, tcc, :]), ln1w_b, ln1b_b, "ln1")
                transpose_chunk_bf(h1T, h1N, tcc)
        else:
            # ---- last layer: only the final token of each batch ----
            # GTl[o, (h, b)] = sum_e M_h[e, o] x_last_b[e]
            GTl = att_p.tile([P, EC, H, BPC], MM_DT, tag="gtl")
            for h in range(H):
                m_t = wtile(w_p, mqk[l, h], "w")
                for oc in range(EC):
                    ps = ppt.tile([P, BPC], F32, tag="tp")
                    for ec in range(EC):
                        _mm(nc, ps[:], m_t[:, ec, oc * P : (oc + 1) * P],
                            xT[:, ec, T - 1 :: T], ec == 0, ec == EC - 1)
                    evac(GTl[:, oc, h, :], ps[:], (h + oc) % 2)
            # energy for all heads at once: [H, T] per batch
            WTt = att_p.tile([P, EC, H, BPC], MM_DT, tag="wtt")
            for b in range(BPC):
                t0 = b * T
                pse = ppa.tile([P, T], F32, tag="ppa")
                for oc in range(EC):
                    _mm(nc, pse[0:H, :], GTl[:, oc, :, b],
                        xT[:, oc, t0 : t0 + T], oc == 0, oc == EC - 1)
                attl = esb_p.tile([P, T], F32, tag="esb", bufs=3)
                denl = st_p.tile([P, 1], F32, tag="den0")
                nc.scalar.activation(
                    out=attl[0:H, :], in_=pse[0:H, :], func=ACTF.Exp,
                    accum_out=denl[0:H],
                )
                recl = st_p.tile([P, 1], F32, tag="rec0")
                nc.vector.reciprocal(out=recl[0:H], in_=denl[0:H])
                attlb = esb_p.tile([P, T], MM_DT, tag="esbl", bufs=1)
                nc.vector.tensor_scalar_mul(
                    out=_c(attlb[0:H, :]), in0=attl[0:H, :], scalar1=recl[0:H]
                )
                # attTl[j, h] per j-chunk
                attTl = esb_p.tile([P, 2, H], MM_DT, tag="attl", bufs=1)
                for jc in range(2):
                    tp = ppt.tile([P, H], MM_DT, tag="tp")
                    nc.tensor.transpose(
                        tp[:], _c(attlb[0:H, jc * P : (jc + 1) * P]),
                        _c(ident_b[0:H, 0:H]) if MODE == "bf16"
                        else ident_t[0:H, 0:H],
                    )
                    evac(attTl[:, jc, :], tp[:], jc % 2)
                # W[h, e] = sum_j att[h, j] x_b[j, e]
                wps = ppb.tile([P, E], F32, tag="ppb")
                _mm(nc, wps[0:H, :], attTl[:, 0, :], xNb[:, 2 * b, :], True, False)
                _mm(nc, wps[0:H, :], attTl[:, 1, :], xNb[:, 2 * b + 1, :], False, True)
                wbf = esb_p.tile([P, E], MM_DT, tag="wbf", bufs=1)
                nc.scalar.copy(_c(wbf[0:H, :]), wps[0:H, :])
                # WTt[e, h] per e-chunk (lhsT for the wvo projection)
                for ec in range(EC):
                    tp = ppt.tile([P, H], MM_DT, tag="tp")
                    nc.tensor.transpose(
                        tp[:], _c(wbf[0:H, ec * P : (ec + 1) * P]),
                        _c(ident_b[0:H, 0:H]) if MODE == "bf16"
                        else ident_t[0:H, 0:H],
                    )
                    evac(WTt[:, ec, :, b], tp[:], ec % 2)
            # out[b, o] = sum_{h,e} W[b,h,e] wvo_h[e,o], PSUM-accumulated
            pw_last = ppb.tile([BPC, E], F32, tag="ppb")
            for h in range(H):
                wvo_t = wtile(w_p, wvo[l, h], "w")
                for ec in range(EC):
                    _mm(nc, pw_last[:], WTt[:, ec, h, :], wvo_t[:, ec, :],
                        h == 0 and ec == 0, h == H - 1 and ec == EC - 1)

        # ---- FFN block ----
        ff1w_t = wtile(wff_p, ff1w[l], "wff1")
        ff2w_t = wtile(wff_p, ff2w[l], "wff2")

        if not last:
            r1T = ff_p.tile([P, EC, TOK], MM_DT, tag="ffT2")
            for fc in range(EC):
                ps = ppb.tile([P, TOK], F32, tag="ppb")
                for ec in range(EC):
                    _mm(nc, ps[:], ff1w_t[:, ec, fc * P : (fc + 1) * P],
                        h1T[:, ec, :], ec == 0, ec == EC - 1)
                if plain:
                    nc.scalar.activation(
                        out=_c(r1T[:, fc, :]), in_=ps[:], func=ACTF.Relu
                    )
                else:
                    nc.scalar.activation(
                        out=_c(r1T[:, fc, :]), in_=ps[:], func=ACTF.Relu,
                        bias=ff1b_t[:, fc : fc + 1],
                    )

            xN_new = act_p.tile([P, TC, E], F32, tag="xN")
            xNb_new = act_p.tile([P, TC, E], BF16, tag="xNb")
            xT_new = act_p.tile([P, EC, TOK], MM_DT, tag="xT")
            for tcc in range(TC):
                ps = ppb.tile([P, E], F32, tag="ppb")
                for fc in range(EC):
                    _mm(nc, ps[:], r1T[:, fc, tcc * P : (tcc + 1) * P],
                        ff2w_t[:, fc, :], fc == 0, fc == EC - 1)
                s2 = tmp_p.tile([P, E], F32, tag="s1")
                if plain:
                    nc.vector.tensor_add(
                        out=s2[:], in0=ps[:], in1=attn_acc[:, tcc, :]
                    )
                else:
                    nc.vector.tensor_add(out=s2[:], in0=ps[:], in1=ff2b_b[:])
                    nc.gpsimd.tensor_add(
                        out=s2[:], in0=s2[:], in1=attn_acc[:, tcc, :]
                    )
                layernorm(s2[:], xN_new[:, tcc, :], ln2w_b, ln2b_b, "ln2")
                if tcc % 2:
                    nc.scalar.copy(xNb_new[:, tcc, :], xN_new[:, tcc, :])
                else:
                    nc.vector.tensor_copy(xNb_new[:, tcc, :], xN_new[:, tcc, :])
                transpose_chunk_bf(xT_new, xNb_new, tcc)
            xN = xN_new
            xNb = xNb_new
            xT = xT_new
        else:
            # ---- last layer FFN on 2 tokens only ----
            x_l = out_p.tile([BPC, E], F32)
            for b in range(BPC):
                nc.sync.dma_start(
                    out=x_l[b : b + 1, :], in_=xN[P - 1 : P, 2 * b + 1, :]
                )
            if plain:
                ao_l = out_p.tile([BPC, E], F32)
                nc.vector.tensor_copy(ao_l[:], pw_last[:])
            else:
                ao_l = out_p.tile([BPC, E], F32)
                nc.vector.tensor_add(
                    out=ao_l[:], in0=pw_last[:], in1=bo_b[:BPC, :]
                )
            s1 = out_p.tile([BPC, E], F32)
            nc.vector.tensor_add(out=s1[:], in0=ao_l[:], in1=x_l[:])
            h1_l = out_p.tile([BPC, E], F32)
            layernorm(s1[:], h1_l[:], ln1w_b, ln1b_b, "lnL1", rows=BPC)
            h1T_l = ff_p.tile([P, EC, BPC], MM_DT, tag="h1Tl")
            for bb in range(EC):
                tp = ppt.tile([P, BPC], F32, tag="tp")
                nc.tensor.transpose(
                    tp[:], h1_l[:, bb * P : (bb + 1) * P],
                    ident_t[0:BPC, 0:BPC],
                )
                evac(h1T_l[:, bb, :], tp[:], bb % 2)
            r1T_l = ff_p.tile([P, EC, BPC], MM_DT, tag="r1Tl")
            for fc in range(EC):
                ps = ppt.tile([P, BPC], F32, tag="tp")
                for ec in range(EC):
                    _mm(nc, ps[:], ff1w_t[:, ec, fc * P : (fc + 1) * P],
                        h1T_l[:, ec, :], ec == 0, ec == EC - 1)
                if plain:
                    nc.scalar.activation(
                        out=_c(r1T_l[:, fc, :]), in_=ps[:], func=ACTF.Relu
                    )
                else:
                    nc.scalar.activation(
                        out=_c(r1T_l[:, fc, :]), in_=ps[:], func=ACTF.Relu,
                        bias=ff1b_t[:, fc : fc + 1],
                    )
            ps2 = ppb.tile([BPC, E], F32, tag="ppb")
            for fc in range(EC):
                _mm(nc, ps2[:], r1T_l[:, fc, :], ff2w_t[:, fc, :],
                    fc == 0, fc == EC - 1)
            s2 = out_p.tile([BPC, E], F32)
            if not plain:
                nc.vector.tensor_add(out=s2[:], in0=ps2[:], in1=ff2b_b[:BPC, :])
                nc.vector.tensor_add(out=s2[:], in0=s2[:], in1=ao_l[:])
            else:
                nc.vector.tensor_add(out=s2[:], in0=ps2[:], in1=ao_l[:])
            xl = out_p.tile([BPC, E], F32)
            layernorm(s2[:], xl[:], ln2w_b, ln2b_b, "lnL2", rows=BPC)
            xlT = ff_p.tile([P, EC, BPC], MM_DT, tag="xlT")
            for bb in range(EC):
                tp = ppt.tile([P, BPC], F32, tag="tp")
                nc.tensor.transpose(
                    tp[:], xl[:, bb * P : (bb + 1) * P],
                    ident_t[0:BPC, 0:BPC],
                )
                evac(xlT[:, bb, :], tp[:], bb % 2)

    # ---- output head: last token of each batch ----
    wout_t = wtile(wff_p, wout, "wff1")
    pl = ppb.tile([BPC, V], F32, tag="ppb")
    if nlayers == L and last_opt:
        xl_lhs = xlT
        cols = slice(0, BPC)
    else:
        xl_lhs = xT
        cols = slice(T - 1, TOK, T)
    for ec in range(EC):
        _mm(nc, pl[:], xl_lhs[:, ec, cols], wout_t[:, ec, :], ec == 0, ec == EC - 1)
    logits = out_p.tile([BPC, V], F32)
    if plain:
        nc.vector.tensor_copy(logits[:], pl[:])
    else:
        bout_t = out_p.tile([BPC, V], F32)
        nc.sync.dma_start(out=bout_t[:], in_=bout.partition_broadcast(BPC))
        nc.vector.tensor_add(out=logits[:], in0=pl[:], in1=bout_t[:])
    nmax = out_p.tile([BPC, 1], F32)
    nc.vector.reduce_max(out=nmax[:], in_=logits[:], axis=AX.X, negate=True)
    den = out_p.tile([BPC, 1], F32)
    nc.scalar.activation(
        out=logits[:], in_=logits[:], func=ACTF.Exp,
        bias=nmax[:, 0:1], accum_out=den[:],
    )
    rec = out_p.tile([BPC, 1], F32)
    nc.vector.reciprocal(out=rec[:], in_=den[:])
    nc.vector.tensor_scalar_mul(out=logits[:], in0=logits[:], scalar1=rec[:])
    nc.sync.dma_start(out=probs, in_=logits[:])


def _pe_table():
    i = np.arange(E, dtype=np.float32)
    rates = (1.0 / np.power(np.float32(10000.0), 2.0 * np.floor(i / 2.0) / E)).astype(
        np.float32
    )
    ang = np.arange(T, dtype=np.float32)[:, None] * rates[None, :]
    pe = np.concatenate([np.sin(ang[:, 0::2]), np.cos(ang[:, 1::2])], axis=-1)
    return np.tile(pe.astype(np.float32), (BPC, 1))  # [TOK, E]


def _masks():
    # packed additive mask [P, 3*P]: block0 = (i0, j0), block1 = (i1, j0),
    # block2 = (i1, j1);  mask[i, j] = NEG where j > i
    m = np.zeros((P, 3 * P), dtype=np.float32)
    p = np.arange(P)
    m[:, 0:P] = np.where(p[None, :] > p[:, None], np.float32(NEG), 0.0)       # i0,j0
    # block1: i in [128,256), j in [0,128): j <= 127 < 128 <= i, never masked
    m[:, 2 * P : 3 * P] = np.where(p[None, :] > p[:, None], np.float32(NEG), 0.0)  # i1,j1
    return m


def _prep_in_maps(
    input_tokens, emb, wq, wk, wv, wo, bo, ln1_w, ln1_b, ln2_w, ln2_b,
    ff1_w, ff1_b, ff2_w, ff2_b, wout, bout,
):
    f = lambda x: np.ascontiguousarray(np.asarray(x, dtype=np.float32))
    w = lambda x: np.ascontiguousarray(np.asarray(x, dtype=np.float32).astype(NP_WDT))
    toks = np.asarray(input_tokens).astype(np.int64)
    wq32 = np.asarray(wq, dtype=np.float32)
    wk32 = np.asarray(wk, dtype=np.float32)
    wv32 = np.asarray(wv, dtype=np.float32)
    wo32 = np.asarray(wo, dtype=np.float32).reshape(L, H, E, E)
    mqk = np.matmul(wq32, wk32.transpose(0, 1, 3, 2))   # [L,H,E,E]
    wvo = np.matmul(wv32, wo32)                         # [L,H,E,E]
    shared = {
        "emb": f(emb), "mqk": w(mqk), "wvo": w(wvo),
        "bo": f(bo), "ln1w": f(ln1_w), "ln1b": f(ln1_b), "ln2w": f(ln2_w),
        "ln2b": f(ln2_b), "ff1w": w(ff1_w), "ff1b": f(ff1_b), "ff2w": w(ff2_w),
        "ff2b": f(ff2_b), "wout": w(wout), "bout": f(bout),
        "pe2": _pe_table(), "masks": _masks(),
        "ident": np.eye(P, dtype=np.float32),
    }
    in_maps = []
    for c in range(NCORES):
        t = toks[c * BPC : (c + 1) * BPC].reshape(TOK)  # [512] flat tokens
        tokarr = np.ascontiguousarray(t.reshape(TC, P).T.astype(np.int32))
        in_maps.append({**shared, "tok": tokarr})
    return in_maps


def _inputs_are_plain(inputs):
    z = lambda k: not np.any(np.asarray(inputs[k], dtype=np.float32))
    o = lambda k: np.all(np.asarray(inputs[k], dtype=np.float32) == 1.0)
    return (z("bo") and z("ln1_b") and z("ln2_b") and z("ff1_b") and z("ff2_b")
            and z("bout") and o("ln1_w") and o("ln2_w"))


def kernel(**inputs):
    plain = _inputs_are_plain(inputs)
    key = f"nc_{plain}"
    if key not in _CACHE:
        _CACHE[key] = _build(plain=plain)
    nc = _CACHE[key]
    in_maps = _prep_in_maps(**inputs)
    res = run_bass_kernel_spmd(nc, in_maps, core_ids=list(range(NCORES)))
    _CACHE["last_results"] = res
    out = np.concatenate([res.results[c]["probs"] for c in range(NCORES)], axis=0)
    return out.astype(np.float32)
